# revision 28
# baseline (speedup 1.0000x reference)
# Bass/Tile kernel for nn_EquiConv (gnn_message_passing, memory-bound).
#
# Math (per edge e), with w2_* path scales and e3nn norms folded into weights:
#   s1 = x1[:, :128], v1[u,m] = x1[:, 128+3u+m], s2 = x2[:,0], v2m = x2[:,1+m]
#   out0 = (s1*s2) @ W1 + sum_m (v1m*v2m) @ W4        [E,128]
#   out1m = (s1*v2m) @ W2 + (v1m*s2) @ W3             [E,64] for m=0,1,2
#   w = F2 @ silu(F1 @ silu(F0 @ fw))                 [E,192]
#   res[:, :128] = out0 * w[:, :128]
#   res[:, 128+3w+m] = out1m[:, w] * w[:, 128+w]
#
# Strategy: edge-data-parallel across 8 cores. Per core, tiles of 256 edges
# (2 blocks of 128). Edge-major prescale (tensor_scalar with per-partition
# scalar = per-edge), PE transposes to feature-major, PSUM-accumulated bf16
# matmuls with stationary weights, per-edge FC weights via 3-layer MLP,
# final elementwise on DVE, PE transpose back to edge-major with strided
# PSUM writes producing the interleaved 1o layout directly.

import numpy as np
import ml_dtypes
from contextlib import ExitStack

import concourse.bass as bass
import concourse.tile as tile
from concourse import bacc, mybir
from concourse.bass_utils import run_bass_kernel_spmd

E_TOTAL = 262144
N_CORES = 8
E_CORE = E_TOTAL // N_CORES   # 32768
TILE_E = 256                  # edges per tile (2 blocks of 128)
M0, M1 = 128, 64
BF16 = mybir.dt.bfloat16
F32 = mybir.dt.float32
# module-level so the sim test can swap in a CoreSim-implemented function
ACT_FN = mybir.ActivationFunctionType.Silu
# debug bisection: 1=evac only, 2=+tp matmuls, 3=+res, 4=+T-out, 5=full
STAGE = 5

INV_SQRT3 = 1.0 / np.sqrt(3.0)
C0 = np.sqrt(1.0 / 192.0)
C1 = np.sqrt(3.0 / 192.0)


def build_nc(e_core=E_CORE, num_devices=N_CORES, repeat=1):
    nc = bacc.Bacc("TRN2", target_bir_lowering=False, debug=False,
                   num_devices=num_devices)
    x1 = nc.dram_tensor("x1", [e_core, 320], F32, kind="ExternalInput").ap()
    x2 = nc.dram_tensor("x2", [e_core, 4], F32, kind="ExternalInput").ap()
    fw = nc.dram_tensor("fw", [e_core, 128], F32, kind="ExternalInput").ap()
    wW1 = nc.dram_tensor("wW1", [128, 128], BF16, kind="ExternalInput").ap()
    wW2 = nc.dram_tensor("wW2", [128, 64], BF16, kind="ExternalInput").ap()
    wW3 = nc.dram_tensor("wW3", [64, 64], BF16, kind="ExternalInput").ap()
    wW4 = nc.dram_tensor("wW4", [64, 128], BF16, kind="ExternalInput").ap()
    wF0 = nc.dram_tensor("wF0", [128, 64], BF16, kind="ExternalInput").ap()
    wF1 = nc.dram_tensor("wF1", [64, 64], BF16, kind="ExternalInput").ap()
    wF2 = nc.dram_tensor("wF2", [64, 192], BF16, kind="ExternalInput").ap()
    ident = nc.dram_tensor("ident", [128, 128], BF16, kind="ExternalInput").ap()
    out = nc.dram_tensor("out", [e_core, 320], F32, kind="ExternalOutput").ap()

    with tile.TileContext(nc) as tc, ExitStack() as ctx:
        _body(ctx, tc, x1, x2, fw,
              dict(wW1=wW1, wW2=wW2, wW3=wW3, wW4=wW4,
                   wF0=wF0, wF1=wF1, wF2=wF2, ident=ident),
              out, e_core, repeat)
    nc.compile()
    return nc


def _body(ctx, tc, x1, x2, fw, w_aps, out, e_core, repeat=1):
    nc = tc.nc
    n_tiles = e_core // TILE_E

    const = ctx.enter_context(tc.tile_pool(name="const", bufs=1))
    cW1 = const.tile([128, 128], BF16)
    cW2 = const.tile([128, 64], BF16)
    c34 = const.tile([128, 128], BF16)   # W3 at [0:64,0:64], W4 at [64:128,0:128]
    cF0 = const.tile([128, 64], BF16)
    cF1 = const.tile([64, 64], BF16)
    cF2 = const.tile([64, 192], BF16)    # F2a = [:, 0:128], F2b = [:, 128:192]
    cId = const.tile([128, 128], BF16)

    nc.sync.dma_start(out=cW1[:], in_=w_aps["wW1"])
    nc.sync.dma_start(out=cW2[:], in_=w_aps["wW2"])
    nc.sync.dma_start(out=c34[0:64, 0:64], in_=w_aps["wW3"])
    nc.sync.dma_start(out=c34[64:128, 0:128], in_=w_aps["wW4"])
    nc.sync.dma_start(out=cF0[:], in_=w_aps["wF0"])
    nc.sync.dma_start(out=cF1[:], in_=w_aps["wF1"])
    nc.sync.dma_start(out=cF2[:], in_=w_aps["wF2"])
    nc.sync.dma_start(out=cId[:], in_=w_aps["ident"])

    inp = ctx.enter_context(tc.tile_pool(name="inp", bufs=3))
    pre = ctx.enter_context(tc.tile_pool(name="pre", bufs=2))
    evac = ctx.enter_context(tc.tile_pool(name="evac", bufs=2))
    fcs = ctx.enter_context(tc.tile_pool(name="fcs", bufs=2))
    ress = ctx.enter_context(tc.tile_pool(name="ress", bufs=2))
    obs = ctx.enter_context(tc.tile_pool(name="obs", bufs=2))

    pt1 = ctx.enter_context(tc.tile_pool(name="pt1", bufs=1, space="PSUM"))
    pt2 = ctx.enter_context(tc.tile_pool(name="pt2", bufs=1, space="PSUM"))
    pm1 = ctx.enter_context(tc.tile_pool(name="pm1", bufs=1, space="PSUM"))
    pm2 = ctx.enter_context(tc.tile_pool(name="pm2", bufs=1, space="PSUM"))
    pm3 = ctx.enter_context(tc.tile_pool(name="pm3", bufs=1, space="PSUM"))
    pfc = ctx.enter_context(tc.tile_pool(name="pfc", bufs=1, space="PSUM"))
    pob = ctx.enter_context(tc.tile_pool(name="pob", bufs=2, space="PSUM"))

    # repeat>1 wraps the whole body in a HW loop — used only for timing runs
    # (device wall-clock isolation); the graded path uses repeat=1 (no loop).
    import contextlib
    loop_cm = tc.For_i(0, repeat, 1) if repeat > 1 else contextlib.nullcontext()
    with loop_cm:
     for t in range(n_tiles):
        e0 = t * TILE_E
        x1s = inp.tile([128, 2, 320], BF16)
        nc.gpsimd.dma_start(
            out=x1s[:], in_=x1[e0:e0 + TILE_E, :].rearrange("(n p) d -> p n d", p=128))
        x2s = inp.tile([128, 2, 4], F32)
        nc.sync.dma_start(
            out=x2s[:], in_=x2[e0:e0 + TILE_E, :].rearrange("(n p) d -> p n d", p=128))
        fws = inp.tile([128, 2, 128], BF16)
        nc.gpsimd.dma_start(
            out=fws[:], in_=fw[e0:e0 + TILE_E, :].rearrange("(n p) d -> p n d", p=128))

        # prescale: pres[:, 0, b, :] = x1*s2 ; pres[:, 1+m, b, :] = x1*v2m
        pres = pre.tile([128, 4, 2, 320], BF16)
        for b in range(2):
            for s in range(4):
                nc.vector.tensor_scalar_mul(
                    pres[:, s, b, :], x1s[:, b, :], x2s[:, b, s:s + 1])

        # transposes to feature-major (PSUM, bf16)
        t1 = pt1.tile([128, 1024], BF16)
        t2 = pt2.tile([128, 1024], BF16)
        for b in range(2):
            o = 128 * b
            nc.tensor.transpose(t1[:, 0 + o:128 + o], pres[:, 0, b, 0:128], cId[:])
            nc.tensor.transpose(t1[:, 256 + o:384 + o], fws[:, b, :], cId[:])
            nc.tensor.transpose(t1[:, 512 + o:640 + o], pres[:, 1, b, 0:128], cId[:])
            nc.tensor.transpose(t1[:, 768 + o:896 + o], pres[:, 2, b, 0:128], cId[:])
            nc.tensor.transpose(t2[:, 0 + o:128 + o], pres[:, 3, b, 0:128], cId[:])
            for m in range(3):
                # QTm (v1m*s2 planar) at rows 0:64, DTm (v1m*v2m) at rows 64:128
                oo = 256 * (m + 1) + o
                nc.tensor.transpose(
                    t2[0:64, oo:oo + 128], pres[:, 0, b, 128 + m:320:3], cId[:])
                nc.tensor.transpose(
                    t2[64:128, oo:oo + 128], pres[:, m + 1, b, 128 + m:320:3],
                    cId[:], tile_position=(0, 64))

        t1sb = evac.tile([128, 1024], BF16)
        nc.vector.tensor_copy(t1sb[:], t1[:])
        t2sb = evac.tile([128, 1024], BF16)
        nc.scalar.copy(t2sb[:], t2[:])

        PT = t1sb[:, 0:256]
        FT = t1sb[:, 256:512]
        RT = [t1sb[:, 512:768], t1sb[:, 768:1024], t2sb[:, 0:256]]
        QT = [t2sb[0:64, 256:512], t2sb[0:64, 512:768], t2sb[0:64, 768:1024]]
        DT = [t2sb[64:128, 256:512], t2sb[64:128, 512:768], t2sb[64:128, 768:1024]]

        if STAGE < 2:
            obsb = obs.tile([128, 640], BF16)
            nc.vector.tensor_copy(obsb[:], t2sb[:, 0:640])
            nc.gpsimd.dma_start(
                out=out[e0:e0 + TILE_E, :].rearrange("(n p) d -> p n d", p=128),
                in_=obsb[:].rearrange("p (n d) -> p n d", n=2))
            continue

        mm1 = pm1.tile([128, 512], F32)   # out0 [128,0:256]; m0 [0:64,256:512]
        mm2 = pm2.tile([128, 512], F32)   # m1 [0:64,0:256]; m2 [0:64,256:512]
        mm3 = pm3.tile([128, 512], F32)   # w0 [128,0:256]; w1 [128,256:512]
        fcb = pfc.tile([128, 512], F32)   # h0 [0:64,0:256]; h1 [0:64,256:512]

        # out0 = W1 over PT (K=128 rows 0:127) + W4 over DTm (K=64 rows 64:127)
        nc.tensor.matmul(mm1[:, 0:256], cW1[:], PT, start=True, stop=False)
        for m in range(3):
            nc.tensor.matmul(mm1[:, 0:256], c34[64:128, 0:128], DT[m],
                             start=False, stop=(m == 2), tile_position=(64, 0))

        # out1m = W2 over RTm + W3 over QTm, all at partitions 0:64
        o1 = [mm1[0:64, 256:512], mm2[0:64, 0:256], mm2[0:64, 256:512]]
        for m in range(3):
            nc.tensor.matmul(o1[m], cW2[:], RT[m], start=True, stop=False)
            nc.tensor.matmul(o1[m], c34[0:64, 0:64], QT[m], start=False, stop=True)

        w0s = fcs.tile([128, 256], F32)
        w1s = fcs.tile([128, 256], F32)
        if STAGE >= 5:
            # FC: h0 -> silu -> h1 -> silu -> w0T/w1T
            nc.tensor.matmul(fcb[0:64, 0:256], cF0[:], FT, start=True, stop=True)
            h0s = fcs.tile([64, 256], BF16)
            nc.scalar.activation(h0s[:], fcb[0:64, 0:256], ACT_FN)
            nc.tensor.matmul(fcb[0:64, 256:512], cF1[:], h0s[:],
                             start=True, stop=True)
            h1s = fcs.tile([64, 256], BF16)
            nc.scalar.activation(h1s[:], fcb[0:64, 256:512], ACT_FN)
            nc.tensor.matmul(mm3[:, 0:256], cF2[:, 0:128], h1s[:],
                             start=True, stop=True)
            nc.tensor.matmul(mm3[0:64, 256:512], cF2[:, 128:192], h1s[:],
                             start=True, stop=True)
            nc.tensor.matmul(mm3[64:128, 256:512], cF2[:, 128:192], h1s[:],
                             start=True, stop=True, tile_position=(0, 64))
            nc.scalar.copy(w0s[:], mm3[:, 0:256])
            nc.scalar.copy(w1s[:], mm3[:, 256:512])
        else:
            nc.vector.memset(w0s[:], 1.0)
            nc.vector.memset(w1s[:], 1.0)

        if STAGE < 3:
            obsb = obs.tile([128, 640], BF16)
            nc.vector.tensor_copy(obsb[:, 0:512], mm1[:, 0:512])
            nc.vector.tensor_copy(obsb[:, 512:640], mm2[:, 0:128])
            nc.gpsimd.dma_start(
                out=out[e0:e0 + TILE_E, :].rearrange("(n p) d -> p n d", p=128),
                in_=obsb[:].rearrange("p (n d) -> p n d", n=2))
            continue

        # res = out * w  (feature-major, bf16 out); res1m all at partitions 0:64
        res0 = ress.tile([128, 256], BF16)
        nc.vector.tensor_mul(res0[:], mm1[:, 0:256], w0s[:])
        res1 = []
        for m in range(3):
            r1t = ress.tile([64, 256], BF16, tag=f"res1_{m}")
            res1.append(r1t)
        nc.vector.tensor_mul(res1[0][:], mm1[0:64, 256:512], w1s[0:64, :])
        nc.vector.tensor_mul(res1[1][:], mm2[0:64, 0:256], w1s[0:64, :])
        nc.vector.tensor_mul(res1[2][:], mm2[0:64, 256:512], w1s[0:64, :])

        if STAGE < 4:
            obsb = obs.tile([128, 640], BF16)
            nc.vector.tensor_copy(obsb[:, 0:256], res0[:])
            nc.vector.tensor_copy(obsb[:, 256:384], res1[0][:, 0:128])
            nc.vector.tensor_copy(obsb[:, 384:512], res1[1][:, 0:128])
            nc.vector.tensor_copy(obsb[:, 512:640], res1[2][:, 0:128])
            nc.gpsimd.dma_start(
                out=out[e0:e0 + TILE_E, :].rearrange("(n p) d -> p n d", p=128),
                in_=obsb[:].rearrange("p (n d) -> p n d", n=2))
            continue

        # transpose back to edge-major, m-planar 1o layout (host interleaves)
        ob = pob.tile([128, 640], BF16)
        for b in range(2):
            o = 320 * b
            ib = 128 * b
            nc.tensor.transpose(ob[:, o:o + 128], res0[:, ib:ib + 128], cId[:])
            for m in range(3):
                nc.tensor.transpose(ob[:, o + 128 + 64 * m:o + 192 + 64 * m],
                                    res1[m][:, ib:ib + 128], cId[0:64, 0:64])

        obsb = obs.tile([128, 640], BF16)
        nc.vector.tensor_copy(obsb[:], ob[:])
        nc.gpsimd.dma_start(
            out=out[e0:e0 + TILE_E, :].rearrange("(n p) d -> p n d", p=128),
            in_=obsb[:].rearrange("p (n d) -> p n d", n=2))


def fold_weights(w1_1, w2_1, w1_2, w2_2, w1_3, w2_3, w1_4, w2_4,
                 fcw0, fcw1, fcw2):
    bf = ml_dtypes.bfloat16
    W1 = (w1_1 * w2_1 * C0).astype(bf)
    W2 = (w1_2 * w2_2 * (C1 * INV_SQRT3)).astype(bf)
    W3 = (w1_3 * w2_3 * (C1 * INV_SQRT3)).astype(bf)
    W4 = (w1_4 * w2_4 * (C0 * INV_SQRT3)).astype(bf)
    F0 = (fcw0 * (1.0 / np.sqrt(128.0))).astype(bf)
    F1 = (fcw1 * 0.125).astype(bf)
    F2 = (fcw2 * 0.125).astype(bf)
    return dict(wW1=W1, wW2=W2, wW3=W3, wW4=W4, wF0=F0, wF1=F1, wF2=F2,
                ident=np.eye(128, dtype=bf))


_nc = None


def prepare_in_maps(fea_in1, fea_in2, fea_weight,
                    w1_1, w2_1, w1_2, w2_2, w1_3, w2_3, w1_4, w2_4,
                    fcw0, fcw1, fcw2):
    wmap = fold_weights(np.asarray(w1_1, np.float32), np.asarray(w2_1, np.float32),
                        np.asarray(w1_2, np.float32), np.asarray(w2_2, np.float32),
                        np.asarray(w1_3, np.float32), np.asarray(w2_3, np.float32),
                        np.asarray(w1_4, np.float32), np.asarray(w2_4, np.float32),
                        np.asarray(fcw0, np.float32), np.asarray(fcw1, np.float32),
                        np.asarray(fcw2, np.float32))
    x1 = np.ascontiguousarray(np.asarray(fea_in1, np.float32))
    x2 = np.ascontiguousarray(np.asarray(fea_in2, np.float32))
    fwv = np.ascontiguousarray(np.asarray(fea_weight, np.float32))

    in_maps = []
    for c in range(N_CORES):
        sl = slice(c * E_CORE, (c + 1) * E_CORE)
        m = dict(x1=x1[sl], x2=x2[sl], fw=fwv[sl])
        m.update(wmap)
        in_maps.append(m)
    return in_maps


def run_spmd(in_maps, **kw):
    global _nc
    if _nc is None:
        _nc = build_nc()
    r = run_bass_kernel_spmd(_nc, in_maps, core_ids=list(range(N_CORES)), **kw)
    planar = np.concatenate([r.results[c]["out"] for c in range(N_CORES)], axis=0)
    return unplanarize(planar), r


def kernel(fea_in1, fea_in2, fea_weight, batch_edge,
           w1_1, w2_1, w1_2, w2_2, w1_3, w2_3, w1_4, w2_4,
           fcw0, fcw1, fcw2):
    in_maps = prepare_in_maps(fea_in1, fea_in2, fea_weight,
                              w1_1, w2_1, w1_2, w2_2, w1_3, w2_3, w1_4, w2_4,
                              fcw0, fcw1, fcw2)
    out, _ = run_spmd(in_maps)
    return out


def unplanarize(planar):
    # device emits 1o part m-planar ([.., m, w]); module layout interleaves
    # as 128+3w+m
    n = planar.shape[0]
    out = np.empty_like(planar)
    out[:, :128] = planar[:, :128]
    out[:, 128:] = planar[:, 128:].reshape(n, 3, 64).transpose(0, 2, 1).reshape(n, 192)
    return out
